# revision 1
# baseline (speedup 1.0000x reference)
"""Trainium2 Bass kernel for nn_NetV2 sparse CNN (submanifold sparse conv net).

Network: scatter 150 active pixels/image to 28x28 grid -> SubMConv3x3(1->32)+BN+ReLU
-> SubMConv3x3(32->64)+BN+ReLU -> SparseConv2x2s2(64->64)+BN+ReLU -> flatten(NCHW)
-> FC(12544->128)+ReLU -> FC(128->10) -> log_softmax.

Design notes:
  * The active-pixel pattern is identical for every image, so each sparse conv
    layer is a fixed gather+matmul structure shared batch-wide.
  * Batch is data-parallel across 8 cores (256 images/core) and lives on the
    matmul free (N) axis; channels/sites live on partitions.
  * BN (inference) folds into conv weights+bias.
  * conv1 (cin=1) is a dense matmul whose host-built operator emits each
    OUTPUT SITE'S NEIGHBOR STACK directly (im2col at conv1 output time), so
    conv2 is a single K=32*n matmul per site into per-site PSUM. conv2's psum
    blocks are arranged per conv3 cell, so conv3 is one K=64*m matmul per
    cell. Every PSUM accumulation group stays on one partition base (hard
    constraint of this walrus build).
  * conv1 contracts over a 32-aligned window of (row-major sorted) sites that
    covers each block's content (sites are spatially local), with the site
    vector X replicated at 5 alignments - one matmul per conv1 output block.
  * Dummy warm-up matmuls + ACT-table preloads run during the input-DMA
    window so the PE HAM clock is at 2.4 GHz when real work starts.
  * Two 256-batch column groups share each PSUM bank, evacuated by one
    fused [128,512] relu+bias op (halves evacuation op count).
  * FC1 only needs rows of fc1_w at active output cells (117 of 196).

All matmul operands are bf16 (fp32 PSUM accumulate); rel err ~2e-4 vs fp32 ref.
"""

import numpy as np
import ml_dtypes

B = 2048
S = 150          # active sites per image
H = W = 28
NCORES = 8
BC = B // NCORES  # batch per core = 256
EPS = 1e-5
BF = ml_dtypes.bfloat16
NWARM = 24       # PE warm-up matmuls

_CACHE = {}


# ---------------------------------------------------------------- metadata ---

def _build_meta(yy, xx):
    """Site graph + placement metadata from the shared active-pixel pattern."""
    order = np.argsort(yy.astype(np.int64) * W + xx)  # row-major spatial sort
    yy_s, xx_s = yy[order], xx[order]
    grid = -np.ones((H, W), np.int64)
    grid[yy_s, xx_s] = np.arange(S)

    # 3x3 pad-1 neighbor lists: per out site i, list of (k, j)
    nbrs = []
    for i in range(S):
        y, x = int(yy_s[i]), int(xx_s[i])
        lst = []
        for ky in range(3):
            for kx in range(3):
                iy, ix = y + ky - 1, x + kx - 1
                if 0 <= iy < H and 0 <= ix < W and grid[iy, ix] >= 0:
                    lst.append((ky * 3 + kx, int(grid[iy, ix])))
        nbrs.append(lst)

    # 2x2 stride-2 cells: cell -> list of (k3, j)
    cellmap = {}
    for j in range(S):
        y, x = int(yy_s[j]), int(xx_s[j])
        cellmap.setdefault((y // 2, x // 2), []).append(((y % 2) * 2 + (x % 2), j))
    cells = sorted(cellmap)
    cellsites = [cellmap[c] for c in cells]
    C2 = len(cells)

    # ---- H1S packing with locality-constrained K windows -------------------
    # Each H1S block holds up to four 32-row slots. A block's *content* sites
    # must fit in one 32-aligned window of <=128 sorted-site indices so conv1
    # can produce the whole block with a single windowed matmul.
    blocks = []   # dict(free=[(base,size)...], content=set(), slots={slot:j})

    def window_ok(content):
        lo, hi = min(content), max(content)
        return (hi // 32 - lo // 32) * 32 + 32 <= 128

    def newblock():
        blocks.append(dict(free=[(0, 128)], content=set(), slots={}))
        return len(blocks) - 1

    def place(size, req_bases, content_sites):
        """Allocate `size` partition rows at an allowed base; return (b, base)."""
        for b, blk in enumerate(blocks):
            if not window_ok(blk["content"] | set(content_sites)):
                continue
            for fi, (fb, fs) in enumerate(blk["free"]):
                for base in req_bases:
                    if base >= fb and base + size <= fb + fs:
                        del blk["free"][fi]
                        if base > fb:
                            blk["free"].append((fb, base - fb))
                        if fb + fs > base + size:
                            blk["free"].append((base + size, fb + fs - base - size))
                        blk["content"].update(content_sites)
                        return b, base
        b = newblock()
        blk = blocks[b]
        base = req_bases[0]
        blk["free"] = [(base + size, 128 - base - size)] if base == 0 else \
            [(0, base), (base + size, 128 - base - size)]
        blk["free"] = [f for f in blk["free"] if f[1] > 0]
        blk["content"].update(content_sites)
        return b, base

    prim = {}
    extra = {}

    def contributors(j):
        # conv1 column for site j sums over j's own 3x3 active neighbors
        return [j2 for _, j2 in nbrs[j]]

    for i in range(S):
        lst = nbrs[i]
        n = len(lst)
        npr = min(n, 4)
        content = sorted({c for _, j in lst[:npr] for c in contributors(j)})
        if npr >= 3:
            bases = [0]
        elif npr == 2:
            bases = [0, 64]
        else:
            bases = [0, 32, 64, 96]
        b, base = place(32 * npr, bases, content)
        prim[i] = (b, base, npr)
        for g in range(npr):
            blocks[b]["slots"][base // 32 + g] = lst[g][1]
        if n == 5:
            eb, ebase = place(32, [0], contributors(lst[4][1]))
            assert ebase == 0
            extra[i] = eb
            blocks[eb]["slots"][0] = lst[4][1]

    nb1 = len(blocks)
    slotmap = {}
    kwin = []   # per block: (a, K)
    for b, blk in enumerate(blocks):
        for s, j in blk["slots"].items():
            slotmap[(b, s)] = j
        lo, hi = min(blk["content"]), max(blk["content"])
        a = lo // 32
        K = (hi // 32 - a + 1) * 32
        assert K <= 128
        kwin.append((a, K))

    # ---- H2S packing: per conv3 cell a stacked site block ------------------
    nb2 = 0

    def newblock2():
        nonlocal nb2
        nb2 += 1
        return nb2 - 1

    site_place = {}
    cell_chunks = [None] * C2
    halfq = []
    for c in range(C2):
        lst = cellsites[c]
        m = len(lst)
        if m >= 2:
            b = newblock2()
            site_place[lst[0][1]] = (b, 0)
            site_place[lst[1][1]] = (b, 1)
            chunks = [(0, 2, b)]
            if m == 3:
                eb = newblock2()
                site_place[lst[2][1]] = (eb, 0)
                chunks.append((0, 1, eb))
                halfq.append((eb, 1))
            cell_chunks[c] = chunks
    for c in range(C2):
        if len(cellsites[c]) != 1:
            continue
        j = cellsites[c][0][1]
        if halfq:
            b, hf = halfq.pop()
        else:
            b = newblock2()
            hf = 0
            halfq.append((b, 1))
        site_place[j] = (b, hf)
        cell_chunks[c] = [(64 * hf, 1, b)]

    # ---- PSUM bank-pairing legality ----------------------------------------
    # HW rule (measured): within one PSUM bank and one output partition half,
    # every matmul during a tile's lifetime must use the same tile_position
    # row base. Orient (swap halves of) H2S blocks so both halves' conv2 prim
    # bases can match across a bank pair, renumber so compatible blocks are
    # adjacent, and record the pair/solo schedule.
    blk2 = [[None, None] for _ in range(nb2)]
    for j, (b2, hf) in site_place.items():
        blk2[b2][hf] = j
    swappable = [True] * nb2
    for c in range(C2):
        if len(cellsites[c]) == 3:
            eb = cell_chunks[c][1][2]
            swappable[eb] = False    # extra site must stay at half 0

    from collections import defaultdict
    groups = defaultdict(lambda: defaultdict(list))
    for t in range(nb2):
        pa, pb = prim[blk2[t][0]][1], prim[blk2[t][1]][1]
        key = tuple(sorted((pa, pb)))
        if swappable[t]:
            if (pa, pb) != key:
                blk2[t] = [blk2[t][1], blk2[t][0]]   # swap halves
            groups[key][key].append(t)
        else:
            groups[key][(pa, pb)].append(t)

    order2 = []
    sched2 = []          # ("pair", col1_block_newid) or ("solo", newid)
    solos = []
    for key in sorted(groups):
        for sig in sorted(groups[key]):
            ts = groups[key][sig]
            while len(ts) >= 2:
                a, b = ts.pop(), ts.pop()
                sched2.append(("pair", len(order2)))
                order2 += [a, b]
            if ts:
                solos.append(ts.pop())
    for t in solos:
        sched2.append(("solo", len(order2)))
        order2.append(t)
    assert len(order2) == nb2
    # renumber: old block id -> new id
    remap2 = {old: new for new, old in enumerate(order2)}
    site_place = {}
    for old, (sA, sB) in enumerate(blk2):
        site_place[sA] = (remap2[old], 0)
        site_place[sB] = (remap2[old], 1)
    cell_chunks = [[(base, m, remap2[b]) for base, m, b in ch]
                   for ch in cell_chunks]
    # m=1 chunks must follow the site's FINAL (post-swap) half
    for c in range(C2):
        if len(cellsites[c]) == 1:
            j = cellsites[c][0][1]
            cell_chunks[c] = [(64 * site_place[j][1], 1, site_place[j][0])]

    # ---- conv3: order cells so bank-paired halves share row bases ----------
    def cellrow(c):
        if len(cellsites[c]) >= 2:
            return 0
        return 64 * site_place[cellsites[c][0][1]][1]

    cellorder = sorted(range(C2), key=cellrow)
    cells = [cells[c] for c in cellorder]
    cellsites = [cellsites[c] for c in cellorder]
    cell_chunks = [cell_chunks[c] for c in cellorder]

    # H3S block signatures -> conv3 bank schedule
    NB3 = (C2 + 1) // 2

    def blk3row(t, h):
        c = 2 * t + h
        return cellrow_new(c) if c < C2 else None

    def cellrow_new(c):
        if len(cellsites[c]) >= 2:
            return 0
        return 64 * site_place[cellsites[c][0][1]][1]

    sched3 = []
    t = 0
    while t < NB3:
        if t + 1 < NB3:
            s0 = (blk3row(t, 0), blk3row(t, 1))
            s1 = (blk3row(t + 1, 0), blk3row(t + 1, 1))
            ok = all(a is None or b is None or a == b for a, b in zip(s0, s1))
            if ok:
                sched3.append(("pair", t))
                t += 2
                continue
        sched3.append(("solo", t))
        t += 1

    # w3stack columns for multi-site cells (in new cell order)
    w3cols = {}
    n = 0
    for c in range(C2):
        if len(cellsites[c]) >= 2:
            w3cols[c] = n
            n += 1

    return dict(order=order, nbrs=nbrs, cells=cells, cellsites=cellsites, C2=C2,
                prim=prim, extra=extra, slotmap=slotmap, nb1=nb1, kwin=kwin,
                site_place=site_place, cell_chunks=cell_chunks, nb2=nb2,
                sched2=sched2, sched3=sched3,
                w3cols=w3cols, nw3=max(1, n))


# ----------------------------------------------------------- device program --

def _legalize_single_wait(bir_bytes):
    """Split instructions with >1 sem-wait into EventSemaphore + instruction.

    The walrus build in this environment supports a single sync-wait slot per
    instruction; Tile emits fused multi-waits. Carry the extra waits on
    standalone EventSemaphore instructions on the same engine (same semantics:
    the engine blocks in order until each condition passes).
    """
    import json as _json
    bir = _json.loads(bir_bytes)
    ctr = 0
    for fn in bir.get("functions", []):
        for blk in fn.get("blocks", []):
            insts = blk.get("instructions")
            if not insts:
                continue
            out = []
            for inst in insts:
                si = inst.get("sync_info")
                waits = (si or {}).get("on_wait") or []
                if len(waits) > 1:
                    for wt in waits[:-1]:
                        ctr += 1
                        out.append({
                            "debug": inst.get("debug", 0),
                            "engine": inst["engine"],
                            "ins": [], "outs": [],
                            "name": f"xw{ctr}_{inst['name']}",
                            "opcode": "EventSemaphore",
                            "sync_info": {"on_update": [], "on_wait": [wt]},
                        })
                    si["on_wait"] = [waits[-1]]
                out.append(inst)
            blk["instructions"] = out
    return _json.dumps(bir).encode()


def _build_program(meta):
    import os
    import concourse.bass as bass
    import concourse.mybir as mybir
    import concourse.tile as tile
    STAGES = int(os.environ.get("KSTAGES", "9"))

    class _Bass(bass.Bass):
        def to_json_bytes(self):
            return _legalize_single_wait(super().to_json_bytes())

    dt = mybir.dt
    f32, bf16 = dt.float32, dt.bfloat16
    Relu = mybir.ActivationFunctionType.Relu
    Exp = mybir.ActivationFunctionType.Exp
    Ln = mybir.ActivationFunctionType.Ln
    add_op = mybir.AluOpType.add
    max_op = mybir.AluOpType.max
    X_axis = mybir.AxisListType.X

    nbrs, cellsites, C2 = meta["nbrs"], meta["cellsites"], meta["C2"]
    prim, extra = meta["prim"], meta["extra"]
    site_place, cell_chunks = meta["site_place"], meta["cell_chunks"]
    kwin, w3cols = meta["kwin"], meta["w3cols"]
    NB1, NB2, NW3 = meta["nb1"], meta["nb2"], meta["nw3"]
    NB3 = (C2 + 1) // 2
    M1 = NB1 * 128
    NT1 = 4                       # t1 operator DMA chunks
    t1csz = [M1 // NT1 + (1 if q < M1 // 128 % NT1 else 0) for q in range(NT1)]
    # chunk boundaries in units of 128-col tiles
    tpb = NB1 // NT1
    t1rng = []
    st = 0
    for q in range(NT1):
        en = NB1 if q == NT1 - 1 else st + tpb
        t1rng.append((st, en))
        st = en

    nc = _Bass()
    p_xc = nc.declare_dram_parameter("xc", [128, 5 * BC], bf16, isOutput=False)
    p_t1 = [nc.declare_dram_parameter(f"t1p{q}", [128, (t1rng[q][1] - t1rng[q][0]) * 128],
                                      bf16, isOutput=False) for q in range(NT1)]
    p_w2s = nc.declare_dram_parameter("w2stack", [128, S * 64], bf16, isOutput=False)
    p_w2 = nc.declare_dram_parameter("w2sb", [128, 9 * 64], bf16, isOutput=False)
    p_w3s = nc.declare_dram_parameter("w3stack", [128, NW3 * 64], bf16, isOutput=False)
    p_w3 = nc.declare_dram_parameter("w3sg", [128, 8 * 64], bf16, isOutput=False)
    p_f1 = nc.declare_dram_parameter("fc1g", [128, NB3 * 128], bf16, isOutput=False)
    p_f2 = nc.declare_dram_parameter("fc2w", [128, 10], f32, isOutput=False)
    p_b1 = nc.declare_dram_parameter("b1t", [128, 1], f32, isOutput=False)
    p_b2 = nc.declare_dram_parameter("b2t", [128, 1], f32, isOutput=False)
    p_b3 = nc.declare_dram_parameter("b3t", [128, 1], f32, isOutput=False)
    p_fb = nc.declare_dram_parameter("fc1bt", [128, 1], f32, isOutput=False)
    p_f2b = nc.declare_dram_parameter("fc2bb", [128, 10], f32, isOutput=False)
    p_out = nc.declare_dram_parameter("out", [BC, 10], f32, isOutput=True)

    with tile.TileContext(nc) as tc:
        with (
            tc.tile_pool(name="consts", bufs=1) as consts,
            tc.tile_pool(name="acts", bufs=1) as acts,
            tc.tile_pool(name="pp", bufs=6, space=bass.MemorySpace.PSUM) as pp,
            tc.tile_pool(name="pfc", bufs=1, space=bass.MemorySpace.PSUM) as pfc,
            tc.tile_pool(name="small", bufs=2) as small,
        ):
            # ---- PE warm-up + ACT table preload during the DMA window -----
            wsrc = consts.tile([128, 256], bf16)
            nc.vector.memset(wsrc, 0.001)
            wps = pp.tile([128, 512], f32, tag="ps")
            for _ in range(NWARM):
                nc.tensor.matmul(wps[:, 0:256], wsrc[:, 0:128], wsrc,
                                 start=True, stop=True)
            wact = small.tile([128, 1], f32, tag="wact")
            nc.scalar.activation(out=wact, in_=wsrc[:, 0:1], func=Relu)
            nc.scalar.activation(out=wact, in_=wact, func=Exp)
            nc.scalar.activation(out=wact, in_=wact, func=Ln)

            xc = consts.tile([128, 5 * BC], bf16)
            t1p = [consts.tile([128, (t1rng[q][1] - t1rng[q][0]) * 128], bf16,
                               tag=f"t1p{q}", name=f"t1p{q}") for q in range(NT1)]
            w2stack = consts.tile([128, S * 64], bf16)
            w2sb = consts.tile([128, 9 * 64], bf16)
            w3stack = consts.tile([128, NW3 * 64], bf16)
            w3sg = consts.tile([128, 8 * 64], bf16)
            fc2w = consts.tile([128, 10], f32)
            b1t = consts.tile([128, 1], f32)
            b2t = consts.tile([128, 1], f32)
            b3t = consts.tile([128, 1], f32)
            fc1bt = consts.tile([128, 1], f32)
            fc2bb = consts.tile([128, 10], f32)

            h1s = acts.tile([128, NB1 * BC], bf16)
            h2s = acts.tile([128, NB2 * BC], bf16)
            h3s = acts.tile([128, NB3 * BC], bf16)
            zt = acts.tile([128, BC], f32)

            nc.sync.dma_start(out=xc, in_=p_xc[:])
            nc.sync.dma_start(out=b1t, in_=p_b1[:])
            for q in range(NT1):
                nc.sync.dma_start(out=t1p[q], in_=p_t1[q][:])
            nc.sync.dma_start(out=w2stack, in_=p_w2s[:])
            nc.sync.dma_start(out=w2sb, in_=p_w2[:])
            nc.sync.dma_start(out=b2t, in_=p_b2[:])
            nc.sync.dma_start(out=w3stack, in_=p_w3s[:])
            nc.sync.dma_start(out=w3sg, in_=p_w3[:])
            nc.sync.dma_start(out=b3t, in_=p_b3[:])
            nc.sync.dma_start(out=fc1bt, in_=p_fb[:])
            nc.sync.dma_start(out=fc2w, in_=p_f2[:])
            nc.sync.dma_start(out=fc2bb, in_=p_f2b[:])
            # fc1g shares the t1p0 slot; its DMA waits for conv1 to drain
            fc1g = consts.tile([128, NB3 * 128], bf16, tag="t1p0")
            nc.sync.dma_start(out=fc1g, in_=p_f1[:])

            def evac(idx, dst, src, bias):
                # dst = relu(src + bias); alternate engines to split the load
                if idx % 2 == 0:
                    nc.scalar.activation(out=dst, in_=src, func=Relu,
                                         bias=bias, scale=1.0)
                else:
                    nc.vector.tensor_scalar(out=dst, in0=src, scalar1=bias,
                                            scalar2=0.0, op0=add_op, op1=max_op)

            def t1ap(b):
                """lhsT AP for conv1 block b from the chunked operator tiles."""
                for q in range(NT1):
                    if t1rng[q][0] <= b < t1rng[q][1]:
                        off = (b - t1rng[q][0]) * 128
                        return t1p[q][:, off:off + 128]
                raise AssertionError

            # ---- conv1: H1S = relu(T1S^T @ Xwin + b1), one MM per block ---
            # K padded to 128 (zero operator rows): K=128 enables the
            # compiler's fast-weight-load path (~4x cheaper LDWEIGHTS)
            for tp in range((NB1 + 1) // 2):
                bs = [b for b in (2 * tp, 2 * tp + 1) if b < NB1]
                ps = pp.tile([128, 512], f32, tag="ps")
                for ci, b in enumerate(bs):
                    a, _K = kwin[b]
                    nc.tensor.matmul(ps[:, ci * 256:ci * 256 + 256],
                                     t1ap(b),
                                     xc[:, a * BC:(a + 1) * BC],
                                     start=True, stop=True,
                                     tile_position=(0, 0))
                evac(tp, h1s[:, bs[0] * BC:(bs[0] + len(bs)) * BC],
                     ps[:, 0:256 * len(bs)], b1t)

            # ---- conv2: one stacked matmul per site (+extra for n=5) ------
            if STAGES < 2:
                nc.vector.memset(h2s, 0.0)
            if STAGES < 3:
                nc.vector.memset(h3s, 0.0)
            blocksites = [[] for _ in range(NB2)]
            for j, (b2, hf) in site_place.items():
                blocksites[b2].append((hf, j))

            def conv2_block(ps, t, ci):
                # K padded to 128: zero weight rows null out the other sites
                # sharing the H1S block, and FWL kicks in
                for hf, i in sorted(blocksites[t]):
                    pb, _base, _npr = prim[i]
                    has_extra = i in extra
                    nc.tensor.matmul(
                        ps[64 * hf:64 * hf + 64, ci * 256:ci * 256 + 256],
                        w2stack[:, i * 64:(i + 1) * 64],
                        h1s[:, pb * BC:(pb + 1) * BC],
                        start=True, stop=not has_extra,
                        tile_position=(0, 64 * hf))
                    if has_extra:
                        k5 = nbrs[i][4][0]
                        nc.tensor.matmul(
                            ps[64 * hf:64 * hf + 64, ci * 256:ci * 256 + 256],
                            w2sb[:, k5 * 64:(k5 + 1) * 64],
                            h1s[:, extra[i] * BC:(extra[i] + 1) * BC],
                            start=False, stop=True,
                            tile_position=(0, 64 * hf))

            if STAGES >= 2:
                for si, (kind, t) in enumerate(meta["sched2"]):
                    if kind == "pair":
                        ps = pp.tile([128, 512], f32, tag="ps")
                        conv2_block(ps, t, 0)
                        conv2_block(ps, t + 1, 1)
                        evac(si, h2s[:, t * BC:(t + 2) * BC], ps, b2t)
                    else:
                        ps = pp.tile([128, 512], f32, tag="ps")
                        conv2_block(ps, t, 0)
                        evac(si, h2s[:, t * BC:(t + 1) * BC],
                             ps[:, 0:256], b2t)

            # ---- conv3: one stacked matmul per cell (+extra for m=3) ------
            def conv3_block(ps, t, ci):
                """Emit cells of H3S block t into psum col ci; return #cells."""
                nn = 0
                for c in (2 * t, 2 * t + 1):
                    if c >= C2:
                        continue
                    hc = c % 2
                    chunks = cell_chunks[c]
                    for idx, (base, m, b2) in enumerate(chunks):
                        # K padded to 128 (zero weight rows; FWL)
                        if m >= 2:
                            wap = w3stack[:, w3cols[c] * 64:(w3cols[c] + 1) * 64]
                        else:
                            lst = cellsites[c]
                            k3 = lst[0][0] if len(lst) == 1 else lst[2][0]
                            hf2 = base // 64
                            wap = w3sg[:, (hf2 * 4 + k3) * 64:
                                       (hf2 * 4 + k3 + 1) * 64]
                        nc.tensor.matmul(
                            ps[64 * hc:64 * hc + 64, ci * 256:ci * 256 + 256],
                            wap,
                            h2s[:, b2 * BC:(b2 + 1) * BC],
                            start=(idx == 0), stop=(idx == len(chunks) - 1),
                            tile_position=(0, 64 * hc))
                    nn += 1
                return nn

            if STAGES >= 3:
                for si, (kind, t) in enumerate(meta["sched3"]):
                    ps = pp.tile([128, 512], f32, tag="ps")
                    if kind == "pair":
                        n0 = conv3_block(ps, t, 0)
                        n1 = conv3_block(ps, t + 1, 1)
                        if n0 == 2 and n1 == 2:
                            evac(si, h3s[:, t * BC:(t + 2) * BC], ps, b3t)
                        else:
                            evac(si, h3s[:64 * n0, t * BC:(t + 1) * BC],
                                 ps[:64 * n0, 0:256], b3t[:64 * n0])
                            evac(si + 1, h3s[:64 * n1, (t + 1) * BC:(t + 2) * BC],
                                 ps[:64 * n1, 256:512], b3t[:64 * n1])
                    else:
                        n0 = conv3_block(ps, t, 0)
                        evac(si, h3s[:64 * n0, t * BC:(t + 1) * BC],
                             ps[:64 * n0, 0:256], b3t[:64 * n0])

            # ---- FC1: z = relu(fc1g^T @ h3s + fc1_b) ----------------------
            psz = pfc.tile([128, BC], f32, tag="psz")
            for t in range(NB3):
                kt = min(128, C2 * 64 - t * 128)
                nc.tensor.matmul(psz, fc1g[:kt, t * 128:(t + 1) * 128],
                                 h3s[:kt, t * BC:(t + 1) * BC],
                                 start=(t == 0), stop=(t == NB3 - 1))
            nc.vector.tensor_scalar(out=zt, in0=psz, scalar1=fc1bt,
                                    scalar2=0.0, op0=add_op, op1=max_op)

            # ---- FC2 + log_softmax (batch on partitions) ------------------
            for hb in range(2):
                psl = pfc.tile([128, 10], f32, tag="psl")
                nc.tensor.matmul(psl, zt[:, hb * 128:(hb + 1) * 128], fc2w,
                                 start=True, stop=True)
                u = small.tile([128, 10], f32, tag="u")
                nc.vector.tensor_add(u, psl, fc2bb)
                mx = small.tile([128, 1], f32, tag="mx")
                nc.vector.reduce_max(out=mx, in_=u, axis=X_axis)
                negm = small.tile([128, 1], f32, tag="negm")
                nc.scalar.mul(negm, mx, -1.0)
                e = small.tile([128, 10], f32, tag="e")
                nc.scalar.activation(out=e, in_=u, func=Exp, bias=negm, scale=1.0)
                sm = small.tile([128, 1], f32, tag="sm")
                nc.vector.reduce_sum(out=sm, in_=e, axis=X_axis)
                ls = small.tile([128, 1], f32, tag="ls")
                nc.scalar.activation(out=ls, in_=sm, func=Ln)
                cc = small.tile([128, 1], f32, tag="cc")
                nc.vector.tensor_sub(cc, negm, ls)
                o = small.tile([128, 10], f32, tag="o")
                nc.vector.tensor_scalar_add(o, u, cc)
                nc.sync.dma_start(out=p_out[hb * 128:(hb + 1) * 128, :], in_=o)

    return nc


# ------------------------------------------------------------------- kernel --

def _fold_bn(w, g, b, m, v):
    s = np.asarray(g, np.float64) / np.sqrt(np.asarray(v, np.float64) + EPS)
    return (np.asarray(w, np.float64) * s).astype(np.float32), \
        (np.asarray(b, np.float64) - np.asarray(m, np.float64) * s).astype(np.float32)


def _host_arrays(meta, w1, g1, b1, m1, v1, w2, g2, b2, m2, v2,
                 w3, g3, b3, m3, v3, fc1_w, fc1_b, fc2_w, fc2_b):
    nbrs, cellsites, C2 = meta["nbrs"], meta["cellsites"], meta["C2"]
    prim, slotmap, kwin = meta["prim"], meta["slotmap"], meta["kwin"]
    NB1, NW3 = meta["nb1"], meta["nw3"]
    NB3 = (C2 + 1) // 2
    M1 = NB1 * 128

    w1f, t1 = _fold_bn(w1, g1, b1, m1, v1)
    w2f, t2 = _fold_bn(w2, g2, b2, m2, v2)
    w3f, t3 = _fold_bn(w3, g3, b3, m3, v3)

    # base conv1 operator columns per site: Tcols[src j, site, ch]
    w1k = w1f.reshape(9, 32)
    Tcols = np.zeros((S, S, 32), np.float32)
    for i in range(S):
        for k, j in nbrs[i]:
            Tcols[j, i] += w1k[k]

    # windowed stacked conv1 operator: block b's rows = sites [32a, 32a+128)
    T1P = np.zeros((128, M1), np.float32)
    for (b, s), j in slotmap.items():
        a, _K = kwin[b]
        cols = slice(b * 128 + s * 32, b * 128 + (s + 1) * 32)
        src = Tcols[32 * a: min(S, 32 * a + 128), j, :]
        T1P[:src.shape[0], cols] = src

    # conv2 stacked weights
    w2k = w2f.reshape(9, 32, 64)
    w2stack = np.zeros((128, S * 64), np.float32)
    for i in range(S):
        _, base, npr = prim[i]
        for g in range(npr):
            k = nbrs[i][g][0]
            w2stack[base + 32 * g: base + 32 * (g + 1),
                    i * 64:(i + 1) * 64] = w2k[k]
    w2sb = np.zeros((128, 9 * 64), np.float32)
    for k in range(9):
        w2sb[0:32, k * 64:(k + 1) * 64] = w2k[k]

    # conv3 weights: single-site table, one variant per (half, offset)
    w3k = w3f.reshape(4, 64, 64)
    w3sg = np.zeros((128, 8 * 64), np.float32)
    for hf in range(2):
        for k in range(4):
            w3sg[64 * hf:64 * hf + 64,
                 (hf * 4 + k) * 64:(hf * 4 + k + 1) * 64] = w3k[k]
    # stacked weights follow the (possibly swapped) half order of the cell's
    # first two sites in the H2S block
    site_place = meta["site_place"]
    w3stack = np.zeros((128, NW3 * 64), np.float32)
    n = 0
    for c in range(C2):
        lst = cellsites[c]
        if len(lst) >= 2:
            for k3, j in lst[:2]:
                hf = site_place[j][1]
                w3stack[64 * hf:64 * hf + 64, n * 64:(n + 1) * 64] = w3k[k3]
            n += 1

    # FC1 rows gathered at active cells, (cell, ch) order, K-chunked
    fc1_w = np.asarray(fc1_w, np.float32)
    cells = meta["cells"]
    rows = np.zeros((NB3 * 128, 128), np.float32)
    for nn_, (cy, cx) in enumerate(cells):
        rows[nn_ * 64:(nn_ + 1) * 64] = fc1_w[np.arange(64) * 196 + cy * 14 + cx]
    fc1g = np.ascontiguousarray(
        rows.reshape(NB3, 128, 128).transpose(1, 0, 2).reshape(128, NB3 * 128))

    arrs = {
        "w2stack": w2stack.astype(BF),
        "w2sb": w2sb.astype(BF),
        "w3stack": w3stack.astype(BF),
        "w3sg": w3sg.astype(BF),
        "fc1g": fc1g.astype(BF),
        "fc2w": np.asarray(fc2_w, np.float32),
        "b1t": np.tile(t1, 4)[:, None].astype(np.float32),
        "b2t": np.tile(t2, 2)[:, None].astype(np.float32),
        "b3t": np.tile(t3, 2)[:, None].astype(np.float32),
        "fc1bt": np.asarray(fc1_b, np.float32)[:, None],
        "fc2bb": np.tile(np.asarray(fc2_b, np.float32), (128, 1)),
    }
    # chunked t1 operator
    NT1 = 4
    tpb = NB1 // NT1
    st = 0
    T1Pb = T1P.astype(BF)
    for q in range(NT1):
        en = NB1 if q == NT1 - 1 else st + tpb
        arrs[f"t1p{q}"] = np.ascontiguousarray(T1Pb[:, st * 128:en * 128])
        st = en
    return arrs


def kernel(features, indices, batch_size, w1, g1, b1, m1, v1,
           w2, g2, b2, m2, v2, w3, g3, b3, m3, v3,
           fc1_w, fc1_b, fc2_w, fc2_b, _trace=False):
    from concourse.bass_utils import run_bass_kernel_spmd

    features = np.asarray(features, np.float32)
    indices = np.asarray(indices, np.int32)
    assert int(batch_size) == B and features.shape[0] == B * S

    assert np.array_equal(indices[:, 0], np.repeat(np.arange(B, dtype=np.int32), S)), \
        "indices must be batch-major"
    assert np.array_equal(indices[:, 1:].reshape(B, S, 2),
                          np.broadcast_to(indices[:S, 1:], (B, S, 2))), \
        "active pattern must be identical across the batch"

    yy, xx = indices[:S, 1].copy(), indices[:S, 2].copy()
    key = (yy.tobytes(), xx.tobytes())
    if key not in _CACHE:
        meta = _build_meta(yy, xx)
        _CACHE[key] = (meta, _build_program(meta))
    meta, nc = _CACHE[key]

    common = _host_arrays(meta, w1, g1, b1, m1, v1, w2, g2, b2, m2, v2,
                          w3, g3, b3, m3, v3, fc1_w, fc1_b, fc2_w, fc2_b)

    # X replicated at five 32-site alignments: copy a = sites [32a, 32a+128)
    XT = features.reshape(B, S)[:, meta["order"]].T  # [S, B]
    Xpad = np.zeros((32 * 4 + 128, B), np.float32)
    Xpad[:S] = XT
    in_maps = []
    for c in range(NCORES):
        m = dict(common)
        xcs = np.zeros((128, 5 * BC), np.float32)
        for a in range(5):
            xcs[:, a * BC:(a + 1) * BC] = Xpad[32 * a:32 * a + 128,
                                               c * BC:(c + 1) * BC]
        m["xc"] = xcs.astype(BF)
        in_maps.append(m)

    res = run_bass_kernel_spmd(nc, in_maps, list(range(NCORES)), trace=_trace)
    global LAST_RESULT
    LAST_RESULT = res
    out = np.concatenate([res.results[c]["out"] for c in range(NCORES)], axis=0)
    return np.asarray(out, np.float32)


LAST_RESULT = None

